# revision 20
# baseline (speedup 1.0000x reference)
"""MoE FFN (E=8 experts, top-2) — expert-parallel Bass/Tile kernel for 8 TRN2 cores.

Strategy:
  - Host computes the (tiny) router: logits = x @ gate_w.T, top-2 per token,
    renormalized weights (= sigmoid of logit differences).  Token n is
    dispatched to cores e1(n), e2(n).
  - All matmul operands are bf16 (PE runs bf16 at the same 1 col/cycle rate as
    float32r, but DMA bytes halve and Fast Weight Load engages, hiding
    LDWEIGHTS).  Accumulation stays fp32 in PSUM; output returns fp32.
  - Capacity C adapts to the actual max expert load (rounded to 16), so no
    fixed-1152 padding compute.  One token block: the whole [H, C] hidden
    fits SBUF in bf16, so w1/w2 stream from HBM exactly once (~23 MB/core).
  - mm1: hT[hc] = gelu(w1.T @ xgT + b1) per 128-row h-chunk, accumulating
    over 8 d-chunks; tokens split into ceil(C/512) column subtiles.
  - mm2 computes the TRANSPOSED output: yT[d, n] = w2[h, d].T @ hT[h, n],
    accumulating over 32 h-chunks, streaming token columns — so the adaptive
    capacity cut applies to both matmuls, and w2 needs no host transpose.
  - Gate weighting + combine happen on host (linear post-op, negligible cost).
  - PE warm-up matmuls on scratch SBUF cover the initial DMA latency and
    release the HAM clock throttle before real work arrives.
"""

import re

import numpy as np
import ml_dtypes

import bass_rust
import concourse.bass as bass
import concourse.mybir as mybir
import concourse.tile as tile
from concourse import bacc, bass_utils

P = 128
D_MODEL = 1024
D_HID = 4096
E = 8
TOP_K = 2
N_CORES = 8

DC = D_MODEL // P          # 8 d-chunks (contraction for mm1)
HC = D_HID // P            # 32 h-chunks
HGW = 1024                 # w1 group tile width (8 h-chunks per group)
NHG = D_HID // HGW         # 4 groups
DCQ = D_MODEL // P         # 8 output d-chunks for mm2

F32 = mybir.dt.float32
BF16 = mybir.dt.bfloat16
NP_BF16 = ml_dtypes.bfloat16

N_WARM = 12                # PE warm-up matmuls (~1.8 us) to cover head DMA
W1LO = 2 * P               # w1 group-0 columns packed into the head tensor


def _subs_for(C):
    """Split C token columns into full 512-wide subtiles (PSUM bank limit is
    512 fp32 columns) plus one small remainder, ordered last.  Within the
    interleaved chain groups the stationary operand is shared, so sub sizes
    don't change PE stream time — but a tiny final sub makes the very last
    evict + store (the kernel tail) near-free."""
    sizes = [512] * (C // 512)
    if C % 512:
        sizes.append(C % 512)
    assert sum(sizes) == C
    return sizes


_tail_patched = False


def _patch_light_tail():
    """Replace Tile's end-of-context machinery (multi-wait drain + two
    all-engine EVSEM barriers + semaphore range-clears, ~10us on HW) with
    single-wait drains on the sync engine covering every logical proc's final
    tick.  The NEFF is executed once per load in this flow, so semaphores
    need not be recycled."""
    global _tail_patched
    if _tail_patched:
        return
    _tail_patched = True

    def _drain_and_barrier(self, tick_clock, wait_clock):
        gc = tick_clock.global_clock
        ticks = eval(re.match(r"VectorClock\((.*)\)", repr(gc)).group(1))
        n = len(ticks)
        for i, v in enumerate(ticks):
            if v > 0:
                vc = bass_rust.VectorClock(
                    [v if j == i else 0 for j in range(n)])
                w = self.nc.sync.drain()
                wait_clock.add_sem_waits(
                    w.ins,
                    bass_rust.ScopedClock({None: vc}),
                    bass_rust.ScopedClock({}),
                )
        popped = self.nc._tile_sem_poison_stack.pop()
        assert popped is self._sem_poison

    tile.TileContext._drain_and_barrier = _drain_and_barrier


def build_nc(C):
    _patch_light_tail()
    SUBS = _subs_for(C)
    nc = bacc.Bacc("TRN2", target_bir_lowering=False, debug=False,
                   num_devices=N_CORES)

    # Inputs, pre-tiled on host into consumption order (all contiguous DMAs):
    #   headt [DC, P, W1LO+SUBS[0]] bf16  per-dc packed first-chain data:
    #         cols [0, W1LO)            = w1[dc*128+p, 0:W1LO]   (group 0, k=0,1)
    #         cols [W1LO, W1LO+SUBS[0]) = Xg[0:SUBS[0], dc*128+p]
    #   xgt  [DC, P, C]        bf16  xgt[dc, p, n] = Xg[n, dc*128+p]
    #                                (device reads only cols SUBS[0]:)
    #   w1t  [NHG, DC, P, HGW] bf16  w1t[hg, dc, p, j] = w1[dc*128+p, hg*1024+j]
    #                                (group 0 reads only cols W1LO:)
    #   w2t  [HC, P, D]        bf16  w2t[hc, p, j] = w2[hc*128+p, j]
    #   b1t  [P, HC]           f32   b1t[p, hc] = b1[hc*128+p]
    # Output:
    #   ygt  [D, C]            bf16  ygt[d, n] = y[n, d]  (gate applied on host)
    headt = nc.dram_tensor("headt", [DC, P, W1LO + SUBS[0]], BF16,
                           kind="ExternalInput")
    xgt = nc.dram_tensor("xgt", [DC, P, C], BF16, kind="ExternalInput")
    w1t = nc.dram_tensor("w1t", [NHG, DC, P, HGW], BF16, kind="ExternalInput")
    w2t = nc.dram_tensor("w2t", [HC, P, D_MODEL], BF16, kind="ExternalInput")
    b1t = nc.dram_tensor("b1t", [P, HC], F32, kind="ExternalInput")
    ygt = nc.dram_tensor("ygt", [D_MODEL, C], BF16, kind="ExternalOutput")

    with tile.TileContext(nc) as tc:
        with (
            tc.tile_pool(name="const", bufs=1) as const,
            tc.tile_pool(name="xg", bufs=1) as xg_pool,
            tc.tile_pool(name="w1", bufs=16) as w1_pool,
            tc.tile_pool(name="w2", bufs=1) as w2_pool,
            tc.tile_pool(name="ht", bufs=1) as ht_pool,
            tc.tile_pool(name="yo", bufs=4) as yo_pool,
            tc.tile_pool(name="ps", bufs=8, space="PSUM") as ps_pool,
        ):
            b1_sb = const.tile([P, HC], F32, name="b1sb")
            nc.sync.dma_start(out=b1_sb[:], in_=b1t[:, :])
            warm = const.tile([P, SUBS[0]], BF16, name="warm")
            wdump = const.tile([P, 8], F32, name="wdump")
            nc.vector.memset(warm[:], 0.0)
            # preload the ACT gelu table while the head DMAs stream
            nc.scalar.activation(wdump[:, :1], warm[:, :1],
                                 mybir.ActivationFunctionType.Gelu, bias=0.0)
            # PE warm-up: releases the HAM clock throttle (~3.4us window) and
            # keeps PE busy until the first real operands land
            ps_w = ps_pool.tile([P, SUBS[0]], F32, name="ps")
            for _ in range(N_WARM):
                nc.tensor.matmul(ps_w[:], lhsT=warm[:, :P],
                                 rhs=warm[:, :SUBS[0]], start=True, stop=True)

            # head DMAs: the first chains (hc=0,1) need only w1 group-0
            # k=0,1 plus the sub-0 token columns — packed per-dc into headt,
            # so 8 DMAs (~1.2 MB) unblock the first 6 real chains.  Then the
            # remaining xg columns and the wide w1 group-0 tail stream in.
            S0 = SUBS[0]
            head_sb = []
            xg_sb = []
            w1_cache = {}
            for dc in range(DC):
                h = xg_pool.tile([P, W1LO + S0], BF16, name=f"head{dc}")
                head_sb.append(h)
                eng = nc.sync if dc % 2 == 0 else nc.scalar
                eng.dma_start(out=h[:], in_=headt[dc, :, :])
            for dc in range(DC):
                t = xg_pool.tile([P, C], BF16, name=f"xg{dc}")
                xg_sb.append(t)
                if C > S0:
                    eng = nc.sync if dc % 2 == 1 else nc.scalar
                    eng.dma_start(out=t[:, S0:], in_=xgt[dc, :, S0:])
            for dc in range(DC):
                w1_sb = w1_pool.tile([P, HGW], BF16, name="w1sb")
                w1_cache[(0, dc)] = w1_sb
                eng = nc.sync if dc % 2 == 0 else nc.scalar
                eng.dma_start(out=w1_sb[:, W1LO:], in_=w1t[0, dc, :, W1LO:])

            def w1_slice(hg, dc, k):
                if hg == 0 and k < W1LO // P:
                    return head_sb[dc][:, k * P:(k + 1) * P]
                return w1_cache[(hg, dc)][:, k * P:(k + 1) * P]

            def xg_slice(dc, si, sub0, SUB):
                if si == 0:
                    return head_sb[dc][:, W1LO:W1LO + SUB]
                return xg_sb[dc][:, sub0:sub0 + SUB]

            w2_sb = [None] * HC

            # ---- mm1: hT[hc] = gelu(w1.T @ xgT + b1), tokens in SUBS cols ----
            ht_tiles = []
            for hc in range(HC):
                hg, k = divmod(hc, HC // NHG)
                # prefetch next w1 group, one tile per hc
                nhg = hg + 1
                if nhg < NHG:
                    w1_sb = w1_pool.tile([P, HGW], BF16, name="w1sb")
                    eng = nc.sync if (nhg + k) % 2 == 0 else nc.scalar
                    eng.dma_start(out=w1_sb[:], in_=w1t[nhg, k, :, :])
                    w1_cache[(nhg, k)] = w1_sb
                # prefetch w2, two tiles per hc through mid-mm1
                if 4 <= hc < 20:
                    for j in range(2):
                        w2i = (hc - 4) * 2 + j
                        t = w2_pool.tile([P, D_MODEL], BF16, name=f"w2sb{w2i}")
                        eng = nc.sync if (w2i % 2) == 0 else nc.scalar
                        eng.dma_start(out=t[:], in_=w2t[w2i, :, :])
                        w2_sb[w2i] = t

                # sub-innermost: each w1 k-slice is loaded into the PE array
                # once and streams all 3 token subtiles (3 PSUM chains open)
                ht = ht_pool.tile([P, C], BF16, name=f"ht{hc}")
                pss = [ps_pool.tile([P, SUB], F32, name="ps")
                       for SUB in SUBS]
                for dc in range(DC):
                    sub0 = 0
                    for si, SUB in enumerate(SUBS):
                        nc.tensor.matmul(
                            pss[si][:],
                            lhsT=w1_slice(hg, dc, k),
                            rhs=xg_slice(dc, si, sub0, SUB),
                            start=(dc == 0),
                            stop=(dc == DC - 1),
                        )
                        sub0 += SUB
                sub0 = 0
                for si, SUB in enumerate(SUBS):
                    nc.scalar.activation(
                        ht[:, sub0:sub0 + SUB], pss[si][:],
                        mybir.ActivationFunctionType.Gelu,
                        bias=b1_sb[:, hc:hc + 1],
                    )
                    sub0 += SUB
                ht_tiles.append(ht)

            # ---- mm2: yT[dq, n] = sum_hc w2[hc, dq].T @ hT[hc, n] ----
            ei = 0
            for dq in range(DCQ):
                pss = [ps_pool.tile([P, SUB], F32, name="ps")
                       for SUB in SUBS]
                for hc in range(HC):
                    sub0 = 0
                    for si, SUB in enumerate(SUBS):
                        nc.tensor.matmul(
                            pss[si][:],
                            lhsT=w2_sb[hc][:, dq * P:(dq + 1) * P],
                            rhs=ht_tiles[hc][:, sub0:sub0 + SUB],
                            start=(hc == 0),
                            stop=(hc == HC - 1),
                        )
                        sub0 += SUB
                sub0 = 0
                for si, SUB in enumerate(SUBS):
                    ps = pss[si]
                    yo = yo_pool.tile([P, SUB], BF16, name="yo")
                    nc.vector.tensor_copy(yo[:], ps[:])
                    half = SUB // 2
                    nc.sync.dma_start(
                        out=ygt[dq * P:(dq + 1) * P, sub0:sub0 + half],
                        in_=yo[:, :half],
                    )
                    nc.scalar.dma_start(
                        out=ygt[dq * P:(dq + 1) * P, sub0 + half:sub0 + SUB],
                        in_=yo[:, half:],
                    )
                    ei += 1
                    sub0 += SUB
    nc.compile()
    return nc


_NC_CACHE = {}
TRACE = False
LAST_RESULTS = None


def _get_nc(C):
    if C not in _NC_CACHE:
        _NC_CACHE[C] = build_nc(C)
    return _NC_CACHE[C]


def kernel(x, gate_w, w1, b1, w2, b2):
    x = np.asarray(x, dtype=np.float32)
    gate_w = np.asarray(gate_w, dtype=np.float32)
    w1 = np.asarray(w1, dtype=np.float32)
    b1 = np.asarray(b1, dtype=np.float32)
    w2 = np.asarray(w2, dtype=np.float32)
    b2 = np.asarray(b2, dtype=np.float32)

    B, T, D = x.shape
    N = B * T
    xf = x.reshape(N, D)

    # ---- router (host; 0.05% of model FLOPs — this is the sharding step) ----
    logits = xf @ gate_w.T                           # [N, E]
    order = np.argsort(-logits, axis=1, kind="stable")
    i1, i2 = order[:, 0], order[:, 1]
    l1 = logits[np.arange(N), i1].astype(np.float64)
    l2 = logits[np.arange(N), i2].astype(np.float64)
    g1 = (1.0 / (1.0 + np.exp(l2 - l1))).astype(np.float32)
    g2 = (1.0 - g1).astype(np.float32)

    # ---- dispatch: gather per-expert tokens, pre-tile, cast to bf16 ----
    idx_per_e = []
    gv_per_e = []
    cnts = []
    for e in range(E):
        sel1 = np.nonzero(i1 == e)[0]
        sel2 = np.nonzero(i2 == e)[0]
        idx = np.concatenate([sel1, sel2])
        gv = np.concatenate([g1[sel1], g2[sel2]])
        idx_per_e.append(idx)
        gv_per_e.append(gv)
        cnts.append(idx.shape[0])

    C = max(512, ((max(cnts) + 15) // 16) * 16)      # capacity, 16-aligned
    S0 = _subs_for(C)[0]

    in_maps = []
    for e in range(E):
        idx = idx_per_e[e]
        cnt = cnts[e]
        xg = np.zeros((C, D), np.float32)
        xg[:cnt] = xf[idx]
        xgt = np.ascontiguousarray(xg.T).astype(NP_BF16).reshape(DC, P, C)
        w1t = np.ascontiguousarray(
            w1[e].reshape(DC, P, NHG, HGW).transpose(2, 0, 1, 3)
        ).astype(NP_BF16)
        w2t = w2[e].reshape(HC, P, D_MODEL).astype(NP_BF16)
        b1t = np.ascontiguousarray(b1[e].reshape(HC, P).T)
        headt = np.concatenate([w1t[0, :, :, :W1LO], xgt[:, :, :S0]], axis=2)
        in_maps.append({"headt": headt, "xgt": xgt, "w1t": w1t,
                        "w2t": w2t, "b1t": b1t})

    nc = _get_nc(C)
    res = bass_utils.run_bass_kernel_spmd(
        nc, in_maps, core_ids=list(range(N_CORES)), trace=TRACE)
    global LAST_RESULTS
    LAST_RESULTS = res

    # ---- combine (host): yT -> gate-weighted scatter-add.  Each token occurs
    # in exactly 2 experts, never twice in one, so fancy-index += is safe ----
    out = np.zeros((N, D), np.float32)
    for e in range(E):
        idx = idx_per_e[e]
        cnt = idx.shape[0]
        ygt = res.results[e]["ygt"][:, :cnt].astype(np.float32)  # [D, cnt]
        out[idx] += (ygt * gv_per_e[e][None, :]).T

    if np.any(b2):
        gate_full = np.zeros((N, E), np.float32)
        gate_full[np.arange(N), i1] = g1
        gate_full[np.arange(N), i2] = g2
        out += gate_full @ b2.reshape(E, D)

    return out.reshape(B, T, D)


# revision 23
# speedup vs baseline: 1.0183x; 1.0183x over previous
"""MoE FFN (E=8 experts, top-2) — expert-parallel Bass/Tile kernel for 8 TRN2 cores.

Strategy:
  - Host computes the (tiny) router: logits = x @ gate_w.T, top-2 per token,
    renormalized weights (= sigmoid of logit differences).  Token n is
    dispatched to cores e1(n), e2(n).
  - All matmul operands are bf16 (PE runs bf16 at the same 1 col/cycle rate as
    float32r, but DMA bytes halve and Fast Weight Load engages, hiding
    LDWEIGHTS).  Accumulation stays fp32 in PSUM; output returns fp32.
  - Capacity C adapts to the actual max expert load (rounded to 16), so no
    fixed-1152 padding compute.  One token block: the whole [H, C] hidden
    fits SBUF in bf16, so w1/w2 stream from HBM exactly once (~23 MB/core).
  - mm1: hT[hc] = gelu(w1.T @ xgT + b1) per 128-row h-chunk, accumulating
    over 8 d-chunks; tokens split into ceil(C/512) column subtiles.
  - mm2 computes the TRANSPOSED output: yT[d, n] = w2[h, d].T @ hT[h, n],
    accumulating over 32 h-chunks, streaming token columns — so the adaptive
    capacity cut applies to both matmuls, and w2 needs no host transpose.
  - Gate weighting + combine happen on host (linear post-op, negligible cost).
  - PE warm-up matmuls on scratch SBUF cover the initial DMA latency and
    release the HAM clock throttle before real work arrives.
"""

import re

import numpy as np
import ml_dtypes

import bass_rust
import concourse.bass as bass
import concourse.mybir as mybir
import concourse.tile as tile
from concourse import bacc, bass_utils

P = 128
D_MODEL = 1024
D_HID = 4096
E = 8
TOP_K = 2
N_CORES = 8

DC = D_MODEL // P          # 8 d-chunks (contraction for mm1)
HC = D_HID // P            # 32 h-chunks
HGW = 1024                 # w1 group tile width (8 h-chunks per group)
NHG = D_HID // HGW         # 4 groups
DCQ = D_MODEL // P         # 8 output d-chunks for mm2

F32 = mybir.dt.float32
BF16 = mybir.dt.bfloat16
NP_BF16 = ml_dtypes.bfloat16

N_WARM = 4                 # PE warm-up matmuls: just enough to cover the gap
W_WARM = 384               # until the first packed-head DMA lands; the real
                           # chains themselves finish warming the HAM clock
W1LO = 2 * P               # w1 group-0 columns packed into the head tensor


def _subs_for(C):
    """Split C token columns into full 512-wide subtiles (PSUM bank limit is
    512 fp32 columns) plus one small remainder, ordered last.  Within the
    interleaved chain groups the stationary operand is shared, so sub sizes
    don't change PE stream time — but a tiny final sub makes the very last
    evict + store (the kernel tail) near-free."""
    sizes = [512] * (C // 512)
    if C % 512:
        sizes.append(C % 512)
    assert sum(sizes) == C
    return sizes


_tail_patched = False


def _patch_light_tail():
    """Replace Tile's end-of-context machinery (multi-wait drain + two
    all-engine EVSEM barriers + semaphore range-clears, ~10us on HW) with
    single-wait drains on the sync engine covering every logical proc's final
    tick.  The NEFF is executed once per load in this flow, so semaphores
    need not be recycled."""
    global _tail_patched
    if _tail_patched:
        return
    _tail_patched = True

    def _drain_and_barrier(self, tick_clock, wait_clock):
        gc = tick_clock.global_clock
        ticks = eval(re.match(r"VectorClock\((.*)\)", repr(gc)).group(1))
        n = len(ticks)
        for i, v in enumerate(ticks):
            if v > 0:
                vc = bass_rust.VectorClock(
                    [v if j == i else 0 for j in range(n)])
                w = self.nc.sync.drain()
                wait_clock.add_sem_waits(
                    w.ins,
                    bass_rust.ScopedClock({None: vc}),
                    bass_rust.ScopedClock({}),
                )
        popped = self.nc._tile_sem_poison_stack.pop()
        assert popped is self._sem_poison

    tile.TileContext._drain_and_barrier = _drain_and_barrier


def build_nc(C):
    _patch_light_tail()
    SUBS = _subs_for(C)
    nc = bacc.Bacc("TRN2", target_bir_lowering=False, debug=False,
                   num_devices=N_CORES)

    # Inputs, pre-tiled on host into consumption order (all contiguous DMAs):
    #   headt [DC, P, W1LO+SUBS[0]] bf16  per-dc packed first-chain data:
    #         cols [0, W1LO)            = w1[dc*128+p, 0:W1LO]   (group 0, k=0,1)
    #         cols [W1LO, W1LO+SUBS[0]) = Xg[0:SUBS[0], dc*128+p]
    #   xgt  [DC, P, C]        bf16  xgt[dc, p, n] = Xg[n, dc*128+p]
    #                                (device reads only cols SUBS[0]:)
    #   w1t  [NHG, DC, P, HGW] bf16  w1t[hg, dc, p, j] = w1[dc*128+p, hg*1024+j]
    #                                (group 0 reads only cols W1LO:)
    #   w2t  [HC, P, D]        bf16  w2t[hc, p, j] = w2[hc*128+p, j]
    #   b1t  [P, HC]           f32   b1t[p, hc] = b1[hc*128+p]
    # Output:
    #   ygt  [D, C]            bf16  ygt[d, n] = y[n, d]  (gate applied on host)
    headt = nc.dram_tensor("headt", [DC, P, W1LO + SUBS[0]], BF16,
                           kind="ExternalInput")
    xgt = nc.dram_tensor("xgt", [DC, P, C], BF16, kind="ExternalInput")
    w1t = nc.dram_tensor("w1t", [NHG, DC, P, HGW], BF16, kind="ExternalInput")
    w2t = nc.dram_tensor("w2t", [HC, P, D_MODEL], BF16, kind="ExternalInput")
    b1t = nc.dram_tensor("b1t", [P, HC], F32, kind="ExternalInput")
    ygt = nc.dram_tensor("ygt", [D_MODEL, C], BF16, kind="ExternalOutput")

    with tile.TileContext(nc) as tc:
        with (
            tc.tile_pool(name="const", bufs=1) as const,
            tc.tile_pool(name="xg", bufs=1) as xg_pool,
            tc.tile_pool(name="w1", bufs=16) as w1_pool,
            tc.tile_pool(name="w2", bufs=1) as w2_pool,
            tc.tile_pool(name="ht", bufs=1) as ht_pool,
            tc.tile_pool(name="yo", bufs=4) as yo_pool,
            tc.tile_pool(name="ps", bufs=8, space="PSUM") as ps_pool,
        ):
            b1_sb = const.tile([P, HC], F32, name="b1sb")
            nc.sync.dma_start(out=b1_sb[:], in_=b1t[:, :])
            warm = const.tile([P, W_WARM], BF16, name="warm")
            wdump = const.tile([P, 8], F32, name="wdump")
            nc.vector.memset(warm[:], 0.0)
            # preload the ACT gelu table while the head DMAs stream
            nc.scalar.activation(wdump[:, :1], warm[:, :1],
                                 mybir.ActivationFunctionType.Gelu, bias=0.0)
            ps_w = ps_pool.tile([P, W_WARM], F32, name="ps")
            for _ in range(N_WARM):
                nc.tensor.matmul(ps_w[:], lhsT=warm[:, :P],
                                 rhs=warm[:], start=True, stop=True)

            # head DMAs: the first chains (hc=0,1) need only w1 group-0
            # k=0,1 plus the sub-0 token columns — packed per-dc into headt,
            # so 8 DMAs (~1.2 MB) unblock the first 6 real chains.  Then the
            # remaining xg columns and the wide w1 group-0 tail stream in.
            S0 = SUBS[0]
            head_sb = []
            xg_sb = []
            w1_cache = {}
            for dc in range(DC):
                h = xg_pool.tile([P, W1LO + S0], BF16, name=f"head{dc}")
                head_sb.append(h)
                eng = nc.sync if dc % 2 == 0 else nc.scalar
                eng.dma_start(out=h[:], in_=headt[dc, :, :])
            for dc in range(DC):
                t = xg_pool.tile([P, C], BF16, name=f"xg{dc}")
                xg_sb.append(t)
                if C > S0:
                    eng = nc.sync if dc % 2 == 1 else nc.scalar
                    eng.dma_start(out=t[:, S0:], in_=xgt[dc, :, S0:])
            for dc in range(DC):
                w1_sb = w1_pool.tile([P, HGW], BF16, name="w1sb")
                w1_cache[(0, dc)] = w1_sb
                eng = nc.sync if dc % 2 == 0 else nc.scalar
                eng.dma_start(out=w1_sb[:, W1LO:], in_=w1t[0, dc, :, W1LO:])

            def w1_slice(hg, dc, k):
                if hg == 0 and k < W1LO // P:
                    return head_sb[dc][:, k * P:(k + 1) * P]
                return w1_cache[(hg, dc)][:, k * P:(k + 1) * P]

            def xg_slice(dc, si, sub0, SUB):
                if si == 0:
                    return head_sb[dc][:, W1LO:W1LO + SUB]
                return xg_sb[dc][:, sub0:sub0 + SUB]

            w2_sb = [None] * HC

            # ---- mm1: hT[hc] = gelu(w1.T @ xgT + b1), tokens in SUBS cols ----
            ht_tiles = []
            for hc in range(HC):
                hg, k = divmod(hc, HC // NHG)
                # mid-stream prefetches go on the sync ring only: the scalar
                # engine runs ~1.5us of gelu per hc iteration, and HWDGE
                # dispatch time on it stalls gelu -> PE blocks on PSUM slots
                nhg = hg + 1
                if nhg < NHG:
                    w1_sb = w1_pool.tile([P, HGW], BF16, name="w1sb")
                    nc.sync.dma_start(out=w1_sb[:], in_=w1t[nhg, k, :, :])
                    w1_cache[(nhg, k)] = w1_sb
                # prefetch w2, two tiles per hc through mid-mm1
                if 4 <= hc < 20:
                    for j in range(2):
                        w2i = (hc - 4) * 2 + j
                        t = w2_pool.tile([P, D_MODEL], BF16, name=f"w2sb{w2i}")
                        nc.sync.dma_start(out=t[:], in_=w2t[w2i, :, :])
                        w2_sb[w2i] = t

                # sub-innermost: each w1 k-slice is loaded into the PE array
                # once and streams all 3 token subtiles (3 PSUM chains open)
                ht = ht_pool.tile([P, C], BF16, name=f"ht{hc}")
                pss = [ps_pool.tile([P, SUB], F32, name="ps")
                       for SUB in SUBS]
                for dc in range(DC):
                    sub0 = 0
                    for si, SUB in enumerate(SUBS):
                        nc.tensor.matmul(
                            pss[si][:],
                            lhsT=w1_slice(hg, dc, k),
                            rhs=xg_slice(dc, si, sub0, SUB),
                            start=(dc == 0),
                            stop=(dc == DC - 1),
                        )
                        sub0 += SUB
                sub0 = 0
                for si, SUB in enumerate(SUBS):
                    nc.scalar.activation(
                        ht[:, sub0:sub0 + SUB], pss[si][:],
                        mybir.ActivationFunctionType.Gelu,
                        bias=b1_sb[:, hc:hc + 1],
                    )
                    sub0 += SUB
                ht_tiles.append(ht)

            # ---- mm2: yT[dq, n] = sum_hc w2[hc, dq].T @ hT[hc, n] ----
            ei = 0
            for dq in range(DCQ):
                pss = [ps_pool.tile([P, SUB], F32, name="ps")
                       for SUB in SUBS]
                for hc in range(HC):
                    sub0 = 0
                    for si, SUB in enumerate(SUBS):
                        nc.tensor.matmul(
                            pss[si][:],
                            lhsT=w2_sb[hc][:, dq * P:(dq + 1) * P],
                            rhs=ht_tiles[hc][:, sub0:sub0 + SUB],
                            start=(hc == 0),
                            stop=(hc == HC - 1),
                        )
                        sub0 += SUB
                sub0 = 0
                for si, SUB in enumerate(SUBS):
                    ps = pss[si]
                    yo = yo_pool.tile([P, SUB], BF16, name="yo")
                    nc.vector.tensor_copy(yo[:], ps[:])
                    half = SUB // 2
                    nc.sync.dma_start(
                        out=ygt[dq * P:(dq + 1) * P, sub0:sub0 + half],
                        in_=yo[:, :half],
                    )
                    nc.scalar.dma_start(
                        out=ygt[dq * P:(dq + 1) * P, sub0 + half:sub0 + SUB],
                        in_=yo[:, half:],
                    )
                    ei += 1
                    sub0 += SUB
    nc.compile()
    return nc


_NC_CACHE = {}
TRACE = False
LAST_RESULTS = None


def _get_nc(C):
    if C not in _NC_CACHE:
        _NC_CACHE[C] = build_nc(C)
    return _NC_CACHE[C]


def kernel(x, gate_w, w1, b1, w2, b2):
    x = np.asarray(x, dtype=np.float32)
    gate_w = np.asarray(gate_w, dtype=np.float32)
    w1 = np.asarray(w1, dtype=np.float32)
    b1 = np.asarray(b1, dtype=np.float32)
    w2 = np.asarray(w2, dtype=np.float32)
    b2 = np.asarray(b2, dtype=np.float32)

    B, T, D = x.shape
    N = B * T
    xf = x.reshape(N, D)

    # ---- router (host; 0.05% of model FLOPs — this is the sharding step) ----
    logits = xf @ gate_w.T                           # [N, E]
    order = np.argsort(-logits, axis=1, kind="stable")
    i1, i2 = order[:, 0], order[:, 1]
    l1 = logits[np.arange(N), i1].astype(np.float64)
    l2 = logits[np.arange(N), i2].astype(np.float64)
    g1 = (1.0 / (1.0 + np.exp(l2 - l1))).astype(np.float32)
    g2 = (1.0 - g1).astype(np.float32)

    # ---- dispatch: gather per-expert tokens, pre-tile, cast to bf16 ----
    idx_per_e = []
    gv_per_e = []
    cnts = []
    for e in range(E):
        sel1 = np.nonzero(i1 == e)[0]
        sel2 = np.nonzero(i2 == e)[0]
        idx = np.concatenate([sel1, sel2])
        gv = np.concatenate([g1[sel1], g2[sel2]])
        idx_per_e.append(idx)
        gv_per_e.append(gv)
        cnts.append(idx.shape[0])

    C = max(512, ((max(cnts) + 15) // 16) * 16)      # capacity, 16-aligned
    S0 = _subs_for(C)[0]

    in_maps = []
    for e in range(E):
        idx = idx_per_e[e]
        cnt = cnts[e]
        xg = np.zeros((C, D), np.float32)
        xg[:cnt] = xf[idx]
        xgt = np.ascontiguousarray(xg.T).astype(NP_BF16).reshape(DC, P, C)
        w1t = np.ascontiguousarray(
            w1[e].reshape(DC, P, NHG, HGW).transpose(2, 0, 1, 3)
        ).astype(NP_BF16)
        w2t = w2[e].reshape(HC, P, D_MODEL).astype(NP_BF16)
        b1t = np.ascontiguousarray(b1[e].reshape(HC, P).T)
        headt = np.concatenate([w1t[0, :, :, :W1LO], xgt[:, :, :S0]], axis=2)
        in_maps.append({"headt": headt, "xgt": xgt, "w1t": w1t,
                        "w2t": w2t, "b1t": b1t})

    nc = _get_nc(C)
    res = bass_utils.run_bass_kernel_spmd(
        nc, in_maps, core_ids=list(range(N_CORES)), trace=TRACE)
    global LAST_RESULTS
    LAST_RESULTS = res

    # ---- combine (host): yT -> gate-weighted scatter-add.  Each token occurs
    # in exactly 2 experts, never twice in one, so fancy-index += is safe ----
    out = np.zeros((N, D), np.float32)
    for e in range(E):
        idx = idx_per_e[e]
        cnt = idx.shape[0]
        ygt = res.results[e]["ygt"][:, :cnt].astype(np.float32)  # [D, cnt]
        out[idx] += (ygt * gv_per_e[e][None, :]).T

    if np.any(b2):
        gate_full = np.zeros((N, E), np.float32)
        gate_full[np.arange(N), i1] = g1
        gate_full[np.arange(N), i2] = g2
        out += gate_full @ b2.reshape(E, D)

    return out.reshape(B, T, D)


# revision 24
# speedup vs baseline: 1.0275x; 1.0091x over previous
"""MoE FFN (E=8 experts, top-2) — expert-parallel Bass/Tile kernel for 8 TRN2 cores.

Strategy:
  - Host computes the (tiny) router: logits = x @ gate_w.T, top-2 per token,
    renormalized weights (= sigmoid of logit differences).  Token n is
    dispatched to cores e1(n), e2(n).
  - All matmul operands are bf16 (PE runs bf16 at the same 1 col/cycle rate as
    float32r, but DMA bytes halve and Fast Weight Load engages, hiding
    LDWEIGHTS).  Accumulation stays fp32 in PSUM; output returns fp32.
  - Capacity C adapts to the actual max expert load (rounded to 16), so no
    fixed-1152 padding compute.  One token block: the whole [H, C] hidden
    fits SBUF in bf16, so w1/w2 stream from HBM exactly once (~23 MB/core).
  - mm1: hT[hc] = gelu(w1.T @ xgT + b1) per 128-row h-chunk, accumulating
    over 8 d-chunks; tokens split into ceil(C/512) column subtiles.
  - mm2 computes the TRANSPOSED output: yT[d, n] = w2[h, d].T @ hT[h, n],
    accumulating over 32 h-chunks, streaming token columns — so the adaptive
    capacity cut applies to both matmuls, and w2 needs no host transpose.
  - Gate weighting + combine happen on host (linear post-op, negligible cost).
  - PE warm-up matmuls on scratch SBUF cover the initial DMA latency and
    release the HAM clock throttle before real work arrives.
"""

import re

import numpy as np
import ml_dtypes

import bass_rust
import concourse.bass as bass
import concourse.mybir as mybir
import concourse.tile as tile
from concourse import bacc, bass_utils

P = 128
D_MODEL = 1024
D_HID = 4096
E = 8
TOP_K = 2
N_CORES = 8

DC = D_MODEL // P          # 8 d-chunks (contraction for mm1)
HC = D_HID // P            # 32 h-chunks
HGW = 1024                 # w1 group tile width (8 h-chunks per group)
NHG = D_HID // HGW         # 4 groups
DCQ = D_MODEL // P         # 8 output d-chunks for mm2

F32 = mybir.dt.float32
BF16 = mybir.dt.bfloat16
NP_BF16 = ml_dtypes.bfloat16

N_WARM = 4                 # PE warm-up matmuls: just enough to cover the gap
W_WARM = 384               # until the first packed-head DMA lands; the real
                           # chains themselves finish warming the HAM clock
W1LO = 2 * P               # w1 group-0 columns packed into the head tensor


def _subs_for(C):
    """Split C token columns into full 512-wide subtiles (PSUM bank limit is
    512 fp32 columns) plus one small remainder, ordered last.  Within the
    interleaved chain groups the stationary operand is shared, so sub sizes
    don't change PE stream time — but a tiny final sub makes the very last
    evict + store (the kernel tail) near-free."""
    sizes = [512] * (C // 512)
    if C % 512:
        sizes.append(C % 512)
    assert sum(sizes) == C
    return sizes


_tail_patched = False


def _patch_light_tail():
    """Replace Tile's end-of-context machinery (multi-wait drain + two
    all-engine EVSEM barriers + semaphore range-clears, ~10us on HW) with
    single-wait drains on the sync engine covering every logical proc's final
    tick.  The NEFF is executed once per load in this flow, so semaphores
    need not be recycled."""
    global _tail_patched
    if _tail_patched:
        return
    _tail_patched = True

    def _drain_and_barrier(self, tick_clock, wait_clock):
        gc = tick_clock.global_clock
        ticks = eval(re.match(r"VectorClock\((.*)\)", repr(gc)).group(1))
        n = len(ticks)
        for i, v in enumerate(ticks):
            if v > 0:
                vc = bass_rust.VectorClock(
                    [v if j == i else 0 for j in range(n)])
                w = self.nc.sync.drain()
                wait_clock.add_sem_waits(
                    w.ins,
                    bass_rust.ScopedClock({None: vc}),
                    bass_rust.ScopedClock({}),
                )
        popped = self.nc._tile_sem_poison_stack.pop()
        assert popped is self._sem_poison

    tile.TileContext._drain_and_barrier = _drain_and_barrier


def build_nc(C):
    _patch_light_tail()
    SUBS = _subs_for(C)
    nc = bacc.Bacc("TRN2", target_bir_lowering=False, debug=False,
                   num_devices=N_CORES)

    # Inputs, pre-tiled on host into consumption order (all contiguous DMAs):
    #   headt [DC, P, W1LO+SUBS[0]] bf16  per-dc packed first-chain data:
    #         cols [0, W1LO)            = w1[dc*128+p, 0:W1LO]   (group 0, k=0,1)
    #         cols [W1LO, W1LO+SUBS[0]) = Xg[0:SUBS[0], dc*128+p]
    #   xgt  [DC, P, C]        bf16  xgt[dc, p, n] = Xg[n, dc*128+p]
    #                                (device reads only cols SUBS[0]:)
    #   w1t  [NHG, DC, P, HGW] bf16  w1t[hg, dc, p, j] = w1[dc*128+p, hg*1024+j]
    #                                (group 0 reads only cols W1LO:)
    #   w2t  [HC, P, D]        bf16  w2t[hc, p, j] = w2[hc*128+p, j]
    #   b1t  [P, HC]           f32   b1t[p, hc] = b1[hc*128+p]
    # Output:
    #   ygt  [D, C]            bf16  ygt[d, n] = y[n, d]  (gate applied on host)
    headt = nc.dram_tensor("headt", [DC, P, W1LO + SUBS[0]], BF16,
                           kind="ExternalInput")
    xgt = nc.dram_tensor("xgt", [DC, P, C], BF16, kind="ExternalInput")
    w1t = nc.dram_tensor("w1t", [NHG, DC, P, HGW], BF16, kind="ExternalInput")
    w2t = nc.dram_tensor("w2t", [HC, P, D_MODEL], BF16, kind="ExternalInput")
    b1t = nc.dram_tensor("b1t", [P, HC], F32, kind="ExternalInput")
    ygt = nc.dram_tensor("ygt", [D_MODEL, C], BF16, kind="ExternalOutput")

    with tile.TileContext(nc) as tc:
        with (
            tc.tile_pool(name="const", bufs=1) as const,
            tc.tile_pool(name="xg", bufs=1) as xg_pool,
            tc.tile_pool(name="w1", bufs=16) as w1_pool,
            tc.tile_pool(name="w2", bufs=1) as w2_pool,
            tc.tile_pool(name="ht", bufs=1) as ht_pool,
            tc.tile_pool(name="yo", bufs=4) as yo_pool,
            tc.tile_pool(name="ps", bufs=8, space="PSUM") as ps_pool,
        ):
            b1_sb = const.tile([P, HC], F32, name="b1sb")
            nc.sync.dma_start(out=b1_sb[:], in_=b1t[:, :])
            warm = const.tile([P, W_WARM], BF16, name="warm")
            wdump = const.tile([P, 8], F32, name="wdump")
            nc.vector.memset(warm[:], 0.0)
            # preload the ACT gelu table while the head DMAs stream
            nc.scalar.activation(wdump[:, :1], warm[:, :1],
                                 mybir.ActivationFunctionType.Gelu, bias=0.0)
            ps_w = ps_pool.tile([P, W_WARM], F32, name="ps")
            for _ in range(N_WARM):
                nc.tensor.matmul(ps_w[:], lhsT=warm[:, :P],
                                 rhs=warm[:], start=True, stop=True)

            # head DMAs: the first chains (hc=0,1) need only w1 group-0
            # k=0,1 plus the sub-0 token columns — packed per-dc into headt,
            # so 8 DMAs (~1.2 MB) unblock the first 6 real chains.  Then the
            # remaining xg columns and the wide w1 group-0 tail stream in.
            S0 = SUBS[0]
            head_sb = []
            xg_sb = []
            w1_cache = {}
            for dc in range(DC):
                h = xg_pool.tile([P, W1LO + S0], BF16, name=f"head{dc}")
                head_sb.append(h)
                eng = nc.sync if dc % 2 == 0 else nc.scalar
                eng.dma_start(out=h[:], in_=headt[dc, :, :])
            for dc in range(DC):
                t = xg_pool.tile([P, C], BF16, name=f"xg{dc}")
                xg_sb.append(t)
                if C > S0:
                    eng = nc.sync if dc % 2 == 1 else nc.scalar
                    eng.dma_start(out=t[:, S0:], in_=xgt[dc, :, S0:])
            for dc in range(DC):
                w1_sb = w1_pool.tile([P, HGW], BF16, name="w1sb")
                w1_cache[(0, dc)] = w1_sb
                eng = nc.sync if dc % 2 == 0 else nc.scalar
                eng.dma_start(out=w1_sb[:, W1LO:], in_=w1t[0, dc, :, W1LO:])

            def w1_slice(hg, dc, k):
                if hg == 0 and k < W1LO // P:
                    return head_sb[dc][:, k * P:(k + 1) * P]
                return w1_cache[(hg, dc)][:, k * P:(k + 1) * P]

            def xg_slice(dc, si, sub0, SUB):
                if si == 0:
                    return head_sb[dc][:, W1LO:W1LO + SUB]
                return xg_sb[dc][:, sub0:sub0 + SUB]

            w2_sb = [None] * HC

            # ---- mm1: hT[hc] = gelu(w1.T @ xgT + b1), tokens in SUBS cols ----
            ht_tiles = []
            for hc in range(HC):
                hg, k = divmod(hc, HC // NHG)
                # mid-stream prefetches go on the sync ring only: the scalar
                # engine runs ~1.5us of gelu per hc iteration, and HWDGE
                # dispatch time on it stalls gelu -> PE blocks on PSUM slots
                nhg = hg + 1
                if nhg < NHG:
                    w1_sb = w1_pool.tile([P, HGW], BF16, name="w1sb")
                    nc.sync.dma_start(out=w1_sb[:], in_=w1t[nhg, k, :, :])
                    w1_cache[(nhg, k)] = w1_sb
                # prefetch w2, two tiles per hc through mid-mm1
                if 4 <= hc < 20:
                    for j in range(2):
                        w2i = (hc - 4) * 2 + j
                        t = w2_pool.tile([P, D_MODEL], BF16, name=f"w2sb{w2i}")
                        nc.sync.dma_start(out=t[:], in_=w2t[w2i, :, :])
                        w2_sb[w2i] = t

                # sub-innermost: each w1 k-slice is loaded into the PE array
                # once and streams all 3 token subtiles (3 PSUM chains open)
                ht = ht_pool.tile([P, C], BF16, name=f"ht{hc}")
                pss = [ps_pool.tile([P, SUB], F32, name="ps")
                       for SUB in SUBS]
                if hc < 2 and len(SUBS) > 1:
                    # head era: the sub-0 chain depends only on the packed
                    # head DMAs — run it alone so the PE streams while the
                    # remaining xg columns are still arriving
                    for dc in range(DC):
                        nc.tensor.matmul(
                            pss[0][:],
                            lhsT=w1_slice(hg, dc, k),
                            rhs=xg_slice(dc, 0, 0, SUBS[0]),
                            start=(dc == 0),
                            stop=(dc == DC - 1),
                        )
                    for dc in range(DC):
                        sub0 = SUBS[0]
                        for si in range(1, len(SUBS)):
                            SUB = SUBS[si]
                            nc.tensor.matmul(
                                pss[si][:],
                                lhsT=w1_slice(hg, dc, k),
                                rhs=xg_slice(dc, si, sub0, SUB),
                                start=(dc == 0),
                                stop=(dc == DC - 1),
                            )
                            sub0 += SUB
                else:
                    for dc in range(DC):
                        sub0 = 0
                        for si, SUB in enumerate(SUBS):
                            nc.tensor.matmul(
                                pss[si][:],
                                lhsT=w1_slice(hg, dc, k),
                                rhs=xg_slice(dc, si, sub0, SUB),
                                start=(dc == 0),
                                stop=(dc == DC - 1),
                            )
                            sub0 += SUB
                sub0 = 0
                for si, SUB in enumerate(SUBS):
                    nc.scalar.activation(
                        ht[:, sub0:sub0 + SUB], pss[si][:],
                        mybir.ActivationFunctionType.Gelu,
                        bias=b1_sb[:, hc:hc + 1],
                    )
                    sub0 += SUB
                ht_tiles.append(ht)

            # ---- mm2: yT[dq, n] = sum_hc w2[hc, dq].T @ hT[hc, n] ----
            ei = 0
            for dq in range(DCQ):
                pss = [ps_pool.tile([P, SUB], F32, name="ps")
                       for SUB in SUBS]
                for hc in range(HC):
                    sub0 = 0
                    for si, SUB in enumerate(SUBS):
                        nc.tensor.matmul(
                            pss[si][:],
                            lhsT=w2_sb[hc][:, dq * P:(dq + 1) * P],
                            rhs=ht_tiles[hc][:, sub0:sub0 + SUB],
                            start=(hc == 0),
                            stop=(hc == HC - 1),
                        )
                        sub0 += SUB
                sub0 = 0
                for si, SUB in enumerate(SUBS):
                    ps = pss[si]
                    yo = yo_pool.tile([P, SUB], BF16, name="yo")
                    nc.vector.tensor_copy(yo[:], ps[:])
                    half = SUB // 2
                    nc.sync.dma_start(
                        out=ygt[dq * P:(dq + 1) * P, sub0:sub0 + half],
                        in_=yo[:, :half],
                    )
                    nc.scalar.dma_start(
                        out=ygt[dq * P:(dq + 1) * P, sub0 + half:sub0 + SUB],
                        in_=yo[:, half:],
                    )
                    ei += 1
                    sub0 += SUB
    nc.compile()
    return nc


_NC_CACHE = {}
TRACE = False
LAST_RESULTS = None


def _get_nc(C):
    if C not in _NC_CACHE:
        _NC_CACHE[C] = build_nc(C)
    return _NC_CACHE[C]


def kernel(x, gate_w, w1, b1, w2, b2):
    x = np.asarray(x, dtype=np.float32)
    gate_w = np.asarray(gate_w, dtype=np.float32)
    w1 = np.asarray(w1, dtype=np.float32)
    b1 = np.asarray(b1, dtype=np.float32)
    w2 = np.asarray(w2, dtype=np.float32)
    b2 = np.asarray(b2, dtype=np.float32)

    B, T, D = x.shape
    N = B * T
    xf = x.reshape(N, D)

    # ---- router (host; 0.05% of model FLOPs — this is the sharding step) ----
    logits = xf @ gate_w.T                           # [N, E]
    order = np.argsort(-logits, axis=1, kind="stable")
    i1, i2 = order[:, 0], order[:, 1]
    l1 = logits[np.arange(N), i1].astype(np.float64)
    l2 = logits[np.arange(N), i2].astype(np.float64)
    g1 = (1.0 / (1.0 + np.exp(l2 - l1))).astype(np.float32)
    g2 = (1.0 - g1).astype(np.float32)

    # ---- dispatch: gather per-expert tokens, pre-tile, cast to bf16 ----
    idx_per_e = []
    gv_per_e = []
    cnts = []
    for e in range(E):
        sel1 = np.nonzero(i1 == e)[0]
        sel2 = np.nonzero(i2 == e)[0]
        idx = np.concatenate([sel1, sel2])
        gv = np.concatenate([g1[sel1], g2[sel2]])
        idx_per_e.append(idx)
        gv_per_e.append(gv)
        cnts.append(idx.shape[0])

    C = max(512, ((max(cnts) + 15) // 16) * 16)      # capacity, 16-aligned
    S0 = _subs_for(C)[0]

    in_maps = []
    for e in range(E):
        idx = idx_per_e[e]
        cnt = cnts[e]
        xg = np.zeros((C, D), np.float32)
        xg[:cnt] = xf[idx]
        xgt = np.ascontiguousarray(xg.T).astype(NP_BF16).reshape(DC, P, C)
        w1t = np.ascontiguousarray(
            w1[e].reshape(DC, P, NHG, HGW).transpose(2, 0, 1, 3)
        ).astype(NP_BF16)
        w2t = w2[e].reshape(HC, P, D_MODEL).astype(NP_BF16)
        b1t = np.ascontiguousarray(b1[e].reshape(HC, P).T)
        headt = np.concatenate([w1t[0, :, :, :W1LO], xgt[:, :, :S0]], axis=2)
        in_maps.append({"headt": headt, "xgt": xgt, "w1t": w1t,
                        "w2t": w2t, "b1t": b1t})

    nc = _get_nc(C)
    res = bass_utils.run_bass_kernel_spmd(
        nc, in_maps, core_ids=list(range(N_CORES)), trace=TRACE)
    global LAST_RESULTS
    LAST_RESULTS = res

    # ---- combine (host): yT -> gate-weighted scatter-add.  Each token occurs
    # in exactly 2 experts, never twice in one, so fancy-index += is safe ----
    out = np.zeros((N, D), np.float32)
    for e in range(E):
        idx = idx_per_e[e]
        cnt = idx.shape[0]
        ygt = res.results[e]["ygt"][:, :cnt].astype(np.float32)  # [D, cnt]
        out[idx] += (ygt * gv_per_e[e][None, :]).T

    if np.any(b2):
        gate_full = np.zeros((N, E), np.float32)
        gate_full[np.arange(N), i1] = g1
        gate_full[np.arange(N), i2] = g2
        out += gate_full @ b2.reshape(E, D)

    return out.reshape(B, T, D)


# revision 25
# speedup vs baseline: 1.0306x; 1.0030x over previous
"""MoE FFN (E=8 experts, top-2) — expert-parallel Bass/Tile kernel for 8 TRN2 cores.

Strategy:
  - Host computes the (tiny) router: logits = x @ gate_w.T, top-2 per token,
    renormalized weights (= sigmoid of logit differences).  Token n is
    dispatched to cores e1(n), e2(n).
  - All matmul operands are bf16 (PE runs bf16 at the same 1 col/cycle rate as
    float32r, but DMA bytes halve and Fast Weight Load engages, hiding
    LDWEIGHTS).  Accumulation stays fp32 in PSUM; output returns fp32.
  - Capacity C adapts to the actual max expert load (rounded to 16), so no
    fixed-1152 padding compute.  One token block: the whole [H, C] hidden
    fits SBUF in bf16, so w1/w2 stream from HBM exactly once (~23 MB/core).
  - mm1: hT[hc] = gelu(w1.T @ xgT + b1) per 128-row h-chunk, accumulating
    over 8 d-chunks; tokens split into ceil(C/512) column subtiles.
  - mm2 computes the TRANSPOSED output: yT[d, n] = w2[h, d].T @ hT[h, n],
    accumulating over 32 h-chunks, streaming token columns — so the adaptive
    capacity cut applies to both matmuls, and w2 needs no host transpose.
  - Gate weighting + combine happen on host (linear post-op, negligible cost).
  - PE warm-up matmuls on scratch SBUF cover the initial DMA latency and
    release the HAM clock throttle before real work arrives.
"""

import re

import numpy as np
import ml_dtypes

import bass_rust
import concourse.bass as bass
import concourse.mybir as mybir
import concourse.tile as tile
from concourse import bacc, bass_utils

P = 128
D_MODEL = 1024
D_HID = 4096
E = 8
TOP_K = 2
N_CORES = 8

DC = D_MODEL // P          # 8 d-chunks (contraction for mm1)
HC = D_HID // P            # 32 h-chunks
HGW = 1024                 # w1 group tile width (8 h-chunks per group)
NHG = D_HID // HGW         # 4 groups
DCQ = D_MODEL // P         # 8 output d-chunks for mm2

F32 = mybir.dt.float32
BF16 = mybir.dt.bfloat16
NP_BF16 = ml_dtypes.bfloat16

N_WARM = 14                # PE warm-up matmuls: cover until the first packed-
W_WARM = 384               # head DMA lands (~12.5us measured); the real
                           # chains themselves finish warming the HAM clock
W1LO = 2 * P               # w1 group-0 columns packed into the head tensor


def _subs_for(C):
    """Split C token columns into full 512-wide subtiles (PSUM bank limit is
    512 fp32 columns) plus one small remainder, ordered last.  Within the
    interleaved chain groups the stationary operand is shared, so sub sizes
    don't change PE stream time — but a tiny final sub makes the very last
    evict + store (the kernel tail) near-free."""
    sizes = [512] * (C // 512)
    if C % 512:
        sizes.append(C % 512)
    assert sum(sizes) == C
    return sizes


_tail_patched = False


def _patch_light_tail():
    """Replace Tile's end-of-context machinery (multi-wait drain + two
    all-engine EVSEM barriers + semaphore range-clears, ~10us on HW) with
    single-wait drains on the sync engine covering every logical proc's final
    tick.  The NEFF is executed once per load in this flow, so semaphores
    need not be recycled."""
    global _tail_patched
    if _tail_patched:
        return
    _tail_patched = True

    def _drain_and_barrier(self, tick_clock, wait_clock):
        gc = tick_clock.global_clock
        ticks = eval(re.match(r"VectorClock\((.*)\)", repr(gc)).group(1))
        n = len(ticks)
        for i, v in enumerate(ticks):
            if v > 0:
                vc = bass_rust.VectorClock(
                    [v if j == i else 0 for j in range(n)])
                w = self.nc.sync.drain()
                wait_clock.add_sem_waits(
                    w.ins,
                    bass_rust.ScopedClock({None: vc}),
                    bass_rust.ScopedClock({}),
                )
        popped = self.nc._tile_sem_poison_stack.pop()
        assert popped is self._sem_poison

    tile.TileContext._drain_and_barrier = _drain_and_barrier


def build_nc(C):
    _patch_light_tail()
    SUBS = _subs_for(C)
    nc = bacc.Bacc("TRN2", target_bir_lowering=False, debug=False,
                   num_devices=N_CORES)

    # Inputs, pre-tiled on host into consumption order (all contiguous DMAs):
    #   headt [DC, P, W1LO+SUBS[0]] bf16  per-dc packed first-chain data:
    #         cols [0, W1LO)            = w1[dc*128+p, 0:W1LO]   (group 0, k=0,1)
    #         cols [W1LO, W1LO+SUBS[0]) = Xg[0:SUBS[0], dc*128+p]
    #   xgt  [DC, P, C]        bf16  xgt[dc, p, n] = Xg[n, dc*128+p]
    #                                (device reads only cols SUBS[0]:)
    #   w1t  [NHG, DC, P, HGW] bf16  w1t[hg, dc, p, j] = w1[dc*128+p, hg*1024+j]
    #                                (group 0 reads only cols W1LO:)
    #   w2t  [HC, P, D]        bf16  w2t[hc, p, j] = w2[hc*128+p, j]
    #   b1t  [P, HC]           f32   b1t[p, hc] = b1[hc*128+p]
    # Output:
    #   ygt  [D, C]            bf16  ygt[d, n] = y[n, d]  (gate applied on host)
    headt = nc.dram_tensor("headt", [DC, P, W1LO + SUBS[0]], BF16,
                           kind="ExternalInput")
    xgt = nc.dram_tensor("xgt", [DC, P, C], BF16, kind="ExternalInput")
    w1t = nc.dram_tensor("w1t", [NHG, DC, P, HGW], BF16, kind="ExternalInput")
    w2t = nc.dram_tensor("w2t", [HC, P, D_MODEL], BF16, kind="ExternalInput")
    b1t = nc.dram_tensor("b1t", [P, HC], F32, kind="ExternalInput")
    ygt = nc.dram_tensor("ygt", [D_MODEL, C], BF16, kind="ExternalOutput")

    with tile.TileContext(nc) as tc:
        with (
            tc.tile_pool(name="const", bufs=1) as const,
            tc.tile_pool(name="xg", bufs=1) as xg_pool,
            tc.tile_pool(name="w1", bufs=16) as w1_pool,
            tc.tile_pool(name="w2", bufs=1) as w2_pool,
            tc.tile_pool(name="ht", bufs=1) as ht_pool,
            tc.tile_pool(name="yo", bufs=4) as yo_pool,
            tc.tile_pool(name="ps", bufs=8, space="PSUM") as ps_pool,
        ):
            b1_sb = const.tile([P, HC], F32, name="b1sb")
            nc.sync.dma_start(out=b1_sb[:], in_=b1t[:, :])
            warm = const.tile([P, W_WARM], BF16, name="warm")
            wdump = const.tile([P, 8], F32, name="wdump")
            nc.vector.memset(warm[:], 0.0)
            # preload the ACT gelu table while the head DMAs stream
            nc.scalar.activation(wdump[:, :1], warm[:, :1],
                                 mybir.ActivationFunctionType.Gelu, bias=0.0)
            ps_w = ps_pool.tile([P, W_WARM], F32, name="ps")
            for _ in range(N_WARM):
                nc.tensor.matmul(ps_w[:], lhsT=warm[:, :P],
                                 rhs=warm[:], start=True, stop=True)

            # head DMAs: the first chains (hc=0,1) need only w1 group-0
            # k=0,1 plus the sub-0 token columns — packed per-dc into headt,
            # so 8 DMAs (~1.2 MB) unblock the first 6 real chains.  Then the
            # remaining xg columns and the wide w1 group-0 tail stream in.
            S0 = SUBS[0]
            head_sb = []
            xg_sb = []
            w1_cache = {}
            for dc in range(DC):
                h = xg_pool.tile([P, W1LO + S0], BF16, name=f"head{dc}")
                head_sb.append(h)
                eng = nc.sync if dc % 2 == 0 else nc.scalar
                eng.dma_start(out=h[:], in_=headt[dc, :, :])
            for dc in range(DC):
                t = xg_pool.tile([P, C], BF16, name=f"xg{dc}")
                xg_sb.append(t)
                if C > S0:
                    eng = nc.sync if dc % 2 == 1 else nc.scalar
                    eng.dma_start(out=t[:, S0:], in_=xgt[dc, :, S0:])
            for dc in range(DC):
                w1_sb = w1_pool.tile([P, HGW], BF16, name="w1sb")
                w1_cache[(0, dc)] = w1_sb
                eng = nc.sync if dc % 2 == 0 else nc.scalar
                eng.dma_start(out=w1_sb[:, W1LO:], in_=w1t[0, dc, :, W1LO:])

            def w1_slice(hg, dc, k):
                if hg == 0 and k < W1LO // P:
                    return head_sb[dc][:, k * P:(k + 1) * P]
                return w1_cache[(hg, dc)][:, k * P:(k + 1) * P]

            def xg_slice(dc, si, sub0, SUB):
                if si == 0:
                    return head_sb[dc][:, W1LO:W1LO + SUB]
                return xg_sb[dc][:, sub0:sub0 + SUB]

            w2_sb = [None] * HC

            # ---- mm1: hT[hc] = gelu(w1.T @ xgT + b1), tokens in SUBS cols ----
            ht_tiles = []
            for hc in range(HC):
                hg, k = divmod(hc, HC // NHG)
                # mid-stream prefetches go on the sync ring only: the scalar
                # engine runs ~1.5us of gelu per hc iteration, and HWDGE
                # dispatch time on it stalls gelu -> PE blocks on PSUM slots
                nhg = hg + 1
                if nhg < NHG:
                    w1_sb = w1_pool.tile([P, HGW], BF16, name="w1sb")
                    nc.sync.dma_start(out=w1_sb[:], in_=w1t[nhg, k, :, :])
                    w1_cache[(nhg, k)] = w1_sb
                # prefetch w2, two tiles per hc through mid-mm1
                if 4 <= hc < 20:
                    for j in range(2):
                        w2i = (hc - 4) * 2 + j
                        t = w2_pool.tile([P, D_MODEL], BF16, name=f"w2sb{w2i}")
                        nc.sync.dma_start(out=t[:], in_=w2t[w2i, :, :])
                        w2_sb[w2i] = t

                # sub-innermost: each w1 k-slice is loaded into the PE array
                # once and streams all 3 token subtiles (3 PSUM chains open)
                ht = ht_pool.tile([P, C], BF16, name=f"ht{hc}")
                pss = [ps_pool.tile([P, SUB], F32, name="ps")
                       for SUB in SUBS]
                if hc < 2 and len(SUBS) > 1:
                    # head era: the sub-0 chain depends only on the packed
                    # head DMAs — run it alone so the PE streams while the
                    # remaining xg columns are still arriving
                    for dc in range(DC):
                        nc.tensor.matmul(
                            pss[0][:],
                            lhsT=w1_slice(hg, dc, k),
                            rhs=xg_slice(dc, 0, 0, SUBS[0]),
                            start=(dc == 0),
                            stop=(dc == DC - 1),
                        )
                    for dc in range(DC):
                        sub0 = SUBS[0]
                        for si in range(1, len(SUBS)):
                            SUB = SUBS[si]
                            nc.tensor.matmul(
                                pss[si][:],
                                lhsT=w1_slice(hg, dc, k),
                                rhs=xg_slice(dc, si, sub0, SUB),
                                start=(dc == 0),
                                stop=(dc == DC - 1),
                            )
                            sub0 += SUB
                else:
                    for dc in range(DC):
                        sub0 = 0
                        for si, SUB in enumerate(SUBS):
                            nc.tensor.matmul(
                                pss[si][:],
                                lhsT=w1_slice(hg, dc, k),
                                rhs=xg_slice(dc, si, sub0, SUB),
                                start=(dc == 0),
                                stop=(dc == DC - 1),
                            )
                            sub0 += SUB
                sub0 = 0
                for si, SUB in enumerate(SUBS):
                    nc.scalar.activation(
                        ht[:, sub0:sub0 + SUB], pss[si][:],
                        mybir.ActivationFunctionType.Gelu,
                        bias=b1_sb[:, hc:hc + 1],
                    )
                    sub0 += SUB
                ht_tiles.append(ht)

            # ---- mm2: yT[dq, n] = sum_hc w2[hc, dq].T @ hT[hc, n] ----
            ei = 0
            for dq in range(DCQ):
                pss = [ps_pool.tile([P, SUB], F32, name="ps")
                       for SUB in SUBS]
                for hc in range(HC):
                    sub0 = 0
                    for si, SUB in enumerate(SUBS):
                        nc.tensor.matmul(
                            pss[si][:],
                            lhsT=w2_sb[hc][:, dq * P:(dq + 1) * P],
                            rhs=ht_tiles[hc][:, sub0:sub0 + SUB],
                            start=(hc == 0),
                            stop=(hc == HC - 1),
                        )
                        sub0 += SUB
                sub0 = 0
                for si, SUB in enumerate(SUBS):
                    ps = pss[si]
                    yo = yo_pool.tile([P, SUB], BF16, name="yo")
                    nc.vector.tensor_copy(yo[:], ps[:])
                    half = SUB // 2
                    nc.sync.dma_start(
                        out=ygt[dq * P:(dq + 1) * P, sub0:sub0 + half],
                        in_=yo[:, :half],
                    )
                    nc.scalar.dma_start(
                        out=ygt[dq * P:(dq + 1) * P, sub0 + half:sub0 + SUB],
                        in_=yo[:, half:],
                    )
                    ei += 1
                    sub0 += SUB
    nc.compile()
    return nc


_NC_CACHE = {}
TRACE = False
LAST_RESULTS = None


def _get_nc(C):
    if C not in _NC_CACHE:
        _NC_CACHE[C] = build_nc(C)
    return _NC_CACHE[C]


def kernel(x, gate_w, w1, b1, w2, b2):
    x = np.asarray(x, dtype=np.float32)
    gate_w = np.asarray(gate_w, dtype=np.float32)
    w1 = np.asarray(w1, dtype=np.float32)
    b1 = np.asarray(b1, dtype=np.float32)
    w2 = np.asarray(w2, dtype=np.float32)
    b2 = np.asarray(b2, dtype=np.float32)

    B, T, D = x.shape
    N = B * T
    xf = x.reshape(N, D)

    # ---- router (host; 0.05% of model FLOPs — this is the sharding step) ----
    logits = xf @ gate_w.T                           # [N, E]
    order = np.argsort(-logits, axis=1, kind="stable")
    i1, i2 = order[:, 0], order[:, 1]
    l1 = logits[np.arange(N), i1].astype(np.float64)
    l2 = logits[np.arange(N), i2].astype(np.float64)
    g1 = (1.0 / (1.0 + np.exp(l2 - l1))).astype(np.float32)
    g2 = (1.0 - g1).astype(np.float32)

    # ---- dispatch: gather per-expert tokens, pre-tile, cast to bf16 ----
    idx_per_e = []
    gv_per_e = []
    cnts = []
    for e in range(E):
        sel1 = np.nonzero(i1 == e)[0]
        sel2 = np.nonzero(i2 == e)[0]
        idx = np.concatenate([sel1, sel2])
        gv = np.concatenate([g1[sel1], g2[sel2]])
        idx_per_e.append(idx)
        gv_per_e.append(gv)
        cnts.append(idx.shape[0])

    C = max(512, ((max(cnts) + 15) // 16) * 16)      # capacity, 16-aligned
    S0 = _subs_for(C)[0]

    in_maps = []
    for e in range(E):
        idx = idx_per_e[e]
        cnt = cnts[e]
        xg = np.zeros((C, D), np.float32)
        xg[:cnt] = xf[idx]
        xgt = np.ascontiguousarray(xg.T).astype(NP_BF16).reshape(DC, P, C)
        w1t = np.ascontiguousarray(
            w1[e].reshape(DC, P, NHG, HGW).transpose(2, 0, 1, 3)
        ).astype(NP_BF16)
        w2t = w2[e].reshape(HC, P, D_MODEL).astype(NP_BF16)
        b1t = np.ascontiguousarray(b1[e].reshape(HC, P).T)
        headt = np.concatenate([w1t[0, :, :, :W1LO], xgt[:, :, :S0]], axis=2)
        in_maps.append({"headt": headt, "xgt": xgt, "w1t": w1t,
                        "w2t": w2t, "b1t": b1t})

    nc = _get_nc(C)
    res = bass_utils.run_bass_kernel_spmd(
        nc, in_maps, core_ids=list(range(N_CORES)), trace=TRACE)
    global LAST_RESULTS
    LAST_RESULTS = res

    # ---- combine (host): yT -> gate-weighted scatter-add.  Each token occurs
    # in exactly 2 experts, never twice in one, so fancy-index += is safe ----
    out = np.zeros((N, D), np.float32)
    for e in range(E):
        idx = idx_per_e[e]
        cnt = idx.shape[0]
        ygt = res.results[e]["ygt"][:, :cnt].astype(np.float32)  # [D, cnt]
        out[idx] += (ygt * gv_per_e[e][None, :]).T

    if np.any(b2):
        gate_full = np.zeros((N, E), np.float32)
        gate_full[np.arange(N), i1] = g1
        gate_full[np.arange(N), i2] = g2
        out += gate_full @ b2.reshape(E, D)

    return out.reshape(B, T, D)


# revision 28
# speedup vs baseline: 1.0307x; 1.0001x over previous
"""MoE FFN (E=8 experts, top-2) — expert-parallel Bass/Tile kernel for 8 TRN2 cores.

Strategy:
  - Host computes the (tiny) router: logits = x @ gate_w.T, top-2 per token,
    renormalized weights (= sigmoid of logit differences).  Token n is
    dispatched to cores e1(n), e2(n).
  - All matmul operands are bf16 (PE runs bf16 at the same 1 col/cycle rate as
    float32r, but DMA bytes halve and Fast Weight Load engages, hiding
    LDWEIGHTS).  Accumulation stays fp32 in PSUM; output returns fp32.
  - Capacity C adapts to the actual max expert load (rounded to 16), so no
    fixed-1152 padding compute.  One token block: the whole [H, C] hidden
    fits SBUF in bf16, so w1/w2 stream from HBM exactly once (~23 MB/core).
  - mm1: hT[hc] = gelu(w1.T @ xgT + b1) per 128-row h-chunk, accumulating
    over 8 d-chunks; tokens split into ceil(C/512) column subtiles.
  - mm2 computes the TRANSPOSED output: yT[d, n] = w2[h, d].T @ hT[h, n],
    accumulating over 32 h-chunks, streaming token columns — so the adaptive
    capacity cut applies to both matmuls, and w2 needs no host transpose.
  - Gate weighting + combine happen on host (linear post-op, negligible cost).
  - PE warm-up matmuls on scratch SBUF cover the initial DMA latency and
    release the HAM clock throttle before real work arrives.
"""

import re

import numpy as np
import ml_dtypes

import bass_rust
import concourse.bass as bass
import concourse.mybir as mybir
import concourse.tile as tile
from concourse import bacc, bass_utils

P = 128
D_MODEL = 1024
D_HID = 4096
E = 8
TOP_K = 2
N_CORES = 8

DC = D_MODEL // P          # 8 d-chunks (contraction for mm1)
HC = D_HID // P            # 32 h-chunks
HGW = 1024                 # w1 group tile width (8 h-chunks per group)
NHG = D_HID // HGW         # 4 groups
DCQ = D_MODEL // P         # 8 output d-chunks for mm2

F32 = mybir.dt.float32
BF16 = mybir.dt.bfloat16
NP_BF16 = ml_dtypes.bfloat16

N_WARM = 8                 # PE warm-up matmuls: cover until the first packed-
W_WARM = 384               # head DMA receipt (~9.5us measured); the real
                           # chains themselves finish warming the HAM clock
W1LO = 2 * P               # w1 group-0 columns packed into the head tensor


def _subs_for(C):
    """Split C token columns into full 512-wide subtiles (PSUM bank limit is
    512 fp32 columns) plus one small remainder, ordered last.  Within the
    interleaved chain groups the stationary operand is shared, so sub sizes
    don't change PE stream time — but a tiny final sub makes the very last
    evict + store (the kernel tail) near-free."""
    sizes = [512] * (C // 512)
    if C % 512:
        sizes.append(C % 512)
    assert sum(sizes) == C
    return sizes


_tail_patched = False


def _patch_light_tail():
    """Replace Tile's end-of-context machinery (multi-wait drain + two
    all-engine EVSEM barriers + semaphore range-clears, ~10us on HW) with
    single-wait drains on the sync engine covering every logical proc's final
    tick.  The NEFF is executed once per load in this flow, so semaphores
    need not be recycled."""
    global _tail_patched
    if _tail_patched:
        return
    _tail_patched = True

    def _drain_and_barrier(self, tick_clock, wait_clock):
        gc = tick_clock.global_clock
        ticks = eval(re.match(r"VectorClock\((.*)\)", repr(gc)).group(1))
        n = len(ticks)
        for i, v in enumerate(ticks):
            if v > 0:
                vc = bass_rust.VectorClock(
                    [v if j == i else 0 for j in range(n)])
                w = self.nc.sync.drain()
                wait_clock.add_sem_waits(
                    w.ins,
                    bass_rust.ScopedClock({None: vc}),
                    bass_rust.ScopedClock({}),
                )
        popped = self.nc._tile_sem_poison_stack.pop()
        assert popped is self._sem_poison

    tile.TileContext._drain_and_barrier = _drain_and_barrier


def build_nc(C):
    _patch_light_tail()
    SUBS = _subs_for(C)
    nc = bacc.Bacc("TRN2", target_bir_lowering=False, debug=False,
                   num_devices=N_CORES)

    # Inputs, pre-tiled on host into consumption order (all contiguous DMAs):
    #   headt [DC, P, W1LO+SUBS[0]] bf16  per-dc packed first-chain data:
    #         cols [0, W1LO)            = w1[dc*128+p, 0:W1LO]   (group 0, k=0,1)
    #         cols [W1LO, W1LO+SUBS[0]) = Xg[0:SUBS[0], dc*128+p]
    #   xgt  [DC, P, C]        bf16  xgt[dc, p, n] = Xg[n, dc*128+p]
    #                                (device reads only cols SUBS[0]:)
    #   w1t  [NHG, DC, P, HGW] bf16  w1t[hg, dc, p, j] = w1[dc*128+p, hg*1024+j]
    #                                (group 0 reads only cols W1LO:)
    #   w2t  [HC, P, D]        bf16  w2t[hc, p, j] = w2[hc*128+p, j]
    #   b1t  [P, HC]           f32   b1t[p, hc] = b1[hc*128+p]
    # Output:
    #   ygt  [D, C]            bf16  ygt[d, n] = y[n, d]  (gate applied on host)
    headt = nc.dram_tensor("headt", [DC, P, W1LO + SUBS[0]], BF16,
                           kind="ExternalInput")
    xgt = nc.dram_tensor("xgt", [DC, P, C], BF16, kind="ExternalInput")
    w1t = nc.dram_tensor("w1t", [NHG, DC, P, HGW], BF16, kind="ExternalInput")
    w2t = nc.dram_tensor("w2t", [HC, P, D_MODEL], BF16, kind="ExternalInput")
    b1t = nc.dram_tensor("b1t", [P, HC], F32, kind="ExternalInput")
    ygt = nc.dram_tensor("ygt", [D_MODEL, C], BF16, kind="ExternalOutput")

    with tile.TileContext(nc) as tc:
        with (
            tc.tile_pool(name="const", bufs=1) as const,
            tc.tile_pool(name="xg", bufs=1) as xg_pool,
            tc.tile_pool(name="w1", bufs=16) as w1_pool,
            tc.tile_pool(name="w2", bufs=1) as w2_pool,
            tc.tile_pool(name="ht", bufs=1) as ht_pool,
            tc.tile_pool(name="yo", bufs=4) as yo_pool,
            tc.tile_pool(name="ps", bufs=8, space="PSUM") as ps_pool,
        ):
            b1_sb = const.tile([P, HC], F32, name="b1sb")
            nc.sync.dma_start(out=b1_sb[:], in_=b1t[:, :])
            warm = const.tile([P, W_WARM], BF16, name="warm")
            wdump = const.tile([P, 8], F32, name="wdump")
            nc.vector.memset(warm[:], 0.0)
            # preload the ACT gelu table while the head DMAs stream
            nc.scalar.activation(wdump[:, :1], warm[:, :1],
                                 mybir.ActivationFunctionType.Gelu, bias=0.0)
            ps_w = ps_pool.tile([P, W_WARM], F32, name="ps")
            for _ in range(N_WARM):
                nc.tensor.matmul(ps_w[:], lhsT=warm[:, :P],
                                 rhs=warm[:], start=True, stop=True)

            # head DMAs: the first chains (hc=0,1) need only w1 group-0
            # k=0,1 plus the sub-0 token columns — packed per-dc into headt,
            # so 8 DMAs (~1.2 MB) unblock the first 6 real chains.  Then the
            # remaining xg columns and the wide w1 group-0 tail stream in.
            S0 = SUBS[0]
            head_sb = []
            xg_sb = []
            w1_cache = {}
            for dc in range(DC):
                h = xg_pool.tile([P, W1LO + S0], BF16, name=f"head{dc}")
                head_sb.append(h)
                eng = nc.sync if dc % 2 == 0 else nc.scalar
                eng.dma_start(out=h[:], in_=headt[dc, :, :])
            for dc in range(DC):
                t = xg_pool.tile([P, C], BF16, name=f"xg{dc}")
                xg_sb.append(t)
                if C > S0:
                    eng = nc.sync if dc % 2 == 1 else nc.scalar
                    eng.dma_start(out=t[:, S0:], in_=xgt[dc, :, S0:])
            for dc in range(DC):
                w1_sb = w1_pool.tile([P, HGW], BF16, name="w1sb")
                w1_cache[(0, dc)] = w1_sb
                eng = nc.sync if dc % 2 == 0 else nc.scalar
                eng.dma_start(out=w1_sb[:, W1LO:], in_=w1t[0, dc, :, W1LO:])

            def w1_slice(hg, dc, k):
                if hg == 0 and k < W1LO // P:
                    return head_sb[dc][:, k * P:(k + 1) * P]
                return w1_cache[(hg, dc)][:, k * P:(k + 1) * P]

            def xg_slice(dc, si, sub0, SUB):
                if si == 0:
                    return head_sb[dc][:, W1LO:W1LO + SUB]
                return xg_sb[dc][:, sub0:sub0 + SUB]

            w2_sb = [None] * HC

            # ---- mm1: hT[hc] = gelu(w1.T @ xgT + b1), tokens in SUBS cols ----
            # Head-era schedule (hc = 0, 1): both sub-0 chains depend only on
            # the packed head DMAs, so run them back-to-back first; the s1/s2
            # groups and hc>=2 then have ~2 chains of slack for the xg-rest
            # and w1-tail DMA completion receipts to land.
            ht_tiles = []
            if len(SUBS) > 1:
                head_pss = {hc: [ps_pool.tile([P, SUB], F32, name="ps")
                                 for SUB in SUBS] for hc in (0, 1)}
                for hc in (0, 1):
                    for dc in range(DC):
                        nc.tensor.matmul(
                            head_pss[hc][0][:],
                            lhsT=w1_slice(0, dc, hc),
                            rhs=xg_slice(dc, 0, 0, SUBS[0]),
                            start=(dc == 0),
                            stop=(dc == DC - 1),
                        )
                for hc in (0, 1):
                    for dc in range(DC):
                        sub0 = SUBS[0]
                        for si in range(1, len(SUBS)):
                            SUB = SUBS[si]
                            nc.tensor.matmul(
                                head_pss[hc][si][:],
                                lhsT=w1_slice(0, dc, hc),
                                rhs=xg_slice(dc, si, sub0, SUB),
                                start=(dc == 0),
                                stop=(dc == DC - 1),
                            )
                            sub0 += SUB
                for hc in (0, 1):
                    # the group-1 w1 prefetches the main loop would have
                    # issued at hc=0,1
                    if 1 < NHG:
                        w1_sb = w1_pool.tile([P, HGW], BF16, name="w1sb")
                        nc.sync.dma_start(out=w1_sb[:], in_=w1t[1, hc, :, :])
                        w1_cache[(1, hc)] = w1_sb
                    ht = ht_pool.tile([P, C], BF16, name=f"ht{hc}")
                    sub0 = 0
                    for si, SUB in enumerate(SUBS):
                        nc.scalar.activation(
                            ht[:, sub0:sub0 + SUB], head_pss[hc][si][:],
                            mybir.ActivationFunctionType.Gelu,
                            bias=b1_sb[:, hc:hc + 1],
                        )
                        sub0 += SUB
                    ht_tiles.append(ht)
                hc_start = 2
            else:
                hc_start = 0
            for hc in range(hc_start, HC):
                hg, k = divmod(hc, HC // NHG)
                # mid-stream prefetches go on the sync ring only: the scalar
                # engine runs ~1.5us of gelu per hc iteration, and HWDGE
                # dispatch time on it stalls gelu -> PE blocks on PSUM slots
                nhg = hg + 1
                if nhg < NHG:
                    w1_sb = w1_pool.tile([P, HGW], BF16, name="w1sb")
                    nc.sync.dma_start(out=w1_sb[:], in_=w1t[nhg, k, :, :])
                    w1_cache[(nhg, k)] = w1_sb
                # prefetch w2, two tiles per hc through mid-mm1
                if 4 <= hc < 20:
                    for j in range(2):
                        w2i = (hc - 4) * 2 + j
                        t = w2_pool.tile([P, D_MODEL], BF16, name=f"w2sb{w2i}")
                        nc.sync.dma_start(out=t[:], in_=w2t[w2i, :, :])
                        w2_sb[w2i] = t

                # sub-innermost: each w1 k-slice is loaded into the PE array
                # once and streams all 3 token subtiles (3 PSUM chains open)
                ht = ht_pool.tile([P, C], BF16, name=f"ht{hc}")
                pss = [ps_pool.tile([P, SUB], F32, name="ps")
                       for SUB in SUBS]
                for dc in range(DC):
                    sub0 = 0
                    for si, SUB in enumerate(SUBS):
                        nc.tensor.matmul(
                            pss[si][:],
                            lhsT=w1_slice(hg, dc, k),
                            rhs=xg_slice(dc, si, sub0, SUB),
                            start=(dc == 0),
                            stop=(dc == DC - 1),
                        )
                        sub0 += SUB
                sub0 = 0
                for si, SUB in enumerate(SUBS):
                    nc.scalar.activation(
                        ht[:, sub0:sub0 + SUB], pss[si][:],
                        mybir.ActivationFunctionType.Gelu,
                        bias=b1_sb[:, hc:hc + 1],
                    )
                    sub0 += SUB
                ht_tiles.append(ht)

            # ---- mm2: yT[dq, n] = sum_hc w2[hc, dq].T @ hT[hc, n] ----
            ei = 0
            for dq in range(DCQ):
                pss = [ps_pool.tile([P, SUB], F32, name="ps")
                       for SUB in SUBS]
                for hc in range(HC):
                    sub0 = 0
                    for si, SUB in enumerate(SUBS):
                        nc.tensor.matmul(
                            pss[si][:],
                            lhsT=w2_sb[hc][:, dq * P:(dq + 1) * P],
                            rhs=ht_tiles[hc][:, sub0:sub0 + SUB],
                            start=(hc == 0),
                            stop=(hc == HC - 1),
                        )
                        sub0 += SUB
                sub0 = 0
                for si, SUB in enumerate(SUBS):
                    ps = pss[si]
                    yo = yo_pool.tile([P, SUB], BF16, name="yo")
                    nc.vector.tensor_copy(yo[:], ps[:])
                    half = SUB // 2
                    nc.sync.dma_start(
                        out=ygt[dq * P:(dq + 1) * P, sub0:sub0 + half],
                        in_=yo[:, :half],
                    )
                    nc.scalar.dma_start(
                        out=ygt[dq * P:(dq + 1) * P, sub0 + half:sub0 + SUB],
                        in_=yo[:, half:],
                    )
                    ei += 1
                    sub0 += SUB
    nc.compile()
    return nc


_NC_CACHE = {}
TRACE = False
LAST_RESULTS = None


def _get_nc(C):
    if C not in _NC_CACHE:
        _NC_CACHE[C] = build_nc(C)
    return _NC_CACHE[C]


def kernel(x, gate_w, w1, b1, w2, b2):
    x = np.asarray(x, dtype=np.float32)
    gate_w = np.asarray(gate_w, dtype=np.float32)
    w1 = np.asarray(w1, dtype=np.float32)
    b1 = np.asarray(b1, dtype=np.float32)
    w2 = np.asarray(w2, dtype=np.float32)
    b2 = np.asarray(b2, dtype=np.float32)

    B, T, D = x.shape
    N = B * T
    xf = x.reshape(N, D)

    # ---- router (host; 0.05% of model FLOPs — this is the sharding step) ----
    logits = xf @ gate_w.T                           # [N, E]
    order = np.argsort(-logits, axis=1, kind="stable")
    i1, i2 = order[:, 0], order[:, 1]
    l1 = logits[np.arange(N), i1].astype(np.float64)
    l2 = logits[np.arange(N), i2].astype(np.float64)
    g1 = (1.0 / (1.0 + np.exp(l2 - l1))).astype(np.float32)
    g2 = (1.0 - g1).astype(np.float32)

    # ---- dispatch: gather per-expert tokens, pre-tile, cast to bf16 ----
    idx_per_e = []
    gv_per_e = []
    cnts = []
    for e in range(E):
        sel1 = np.nonzero(i1 == e)[0]
        sel2 = np.nonzero(i2 == e)[0]
        idx = np.concatenate([sel1, sel2])
        gv = np.concatenate([g1[sel1], g2[sel2]])
        idx_per_e.append(idx)
        gv_per_e.append(gv)
        cnts.append(idx.shape[0])

    C = max(512, ((max(cnts) + 15) // 16) * 16)      # capacity, 16-aligned
    S0 = _subs_for(C)[0]

    in_maps = []
    for e in range(E):
        idx = idx_per_e[e]
        cnt = cnts[e]
        xg = np.zeros((C, D), np.float32)
        xg[:cnt] = xf[idx]
        xgt = np.ascontiguousarray(xg.T).astype(NP_BF16).reshape(DC, P, C)
        w1t = np.ascontiguousarray(
            w1[e].reshape(DC, P, NHG, HGW).transpose(2, 0, 1, 3)
        ).astype(NP_BF16)
        w2t = w2[e].reshape(HC, P, D_MODEL).astype(NP_BF16)
        b1t = np.ascontiguousarray(b1[e].reshape(HC, P).T)
        headt = np.concatenate([w1t[0, :, :, :W1LO], xgt[:, :, :S0]], axis=2)
        in_maps.append({"headt": headt, "xgt": xgt, "w1t": w1t,
                        "w2t": w2t, "b1t": b1t})

    nc = _get_nc(C)
    res = bass_utils.run_bass_kernel_spmd(
        nc, in_maps, core_ids=list(range(N_CORES)), trace=TRACE)
    global LAST_RESULTS
    LAST_RESULTS = res

    # ---- combine (host): yT -> gate-weighted scatter-add.  Each token occurs
    # in exactly 2 experts, never twice in one, so fancy-index += is safe ----
    out = np.zeros((N, D), np.float32)
    for e in range(E):
        idx = idx_per_e[e]
        cnt = idx.shape[0]
        ygt = res.results[e]["ygt"][:, :cnt].astype(np.float32)  # [D, cnt]
        out[idx] += (ygt * gv_per_e[e][None, :]).T

    if np.any(b2):
        gate_full = np.zeros((N, E), np.float32)
        gate_full[np.arange(N), i1] = g1
        gate_full[np.arange(N), i2] = g2
        out += gate_full @ b2.reshape(E, D)

    return out.reshape(B, T, D)
